# revision 26
# baseline (speedup 1.0000x reference)
"""MoE (8 experts, top-2) TRN2 kernel — routed expert-parallel variant.

Sharding strategy (host = the shard/unshard glue): compute the top-2 routing
decision on host and shard tokens by expert id — core i receives exactly the
tokens routed to expert i (gathered, bf16, transposed), padded to a common
capacity C. Each core then computes, ON DEVICE, the gating softmax for its
tokens (to get the combine weight = raw softmax prob of its own expert), the
FFN in bf16, scales rows by the combine weight and writes y_part [C, H] in
bf16. Host scatter-adds the per-expert partials back to token order.

Gating prologue (high priority): per chunk, logits land in PSUM, get
transposed token-major and bias-added into one big tile; then a SINGLE Exp
activation covers all chunks (max 3 activation-table loads per run no matter
how the scheduler interleaves), comb = 1/sum_j exp(l_j - l_0) with gating
columns permuted per core so "my expert" is column 0.

DMA layout: x is chunk-major ([128, g, c, 512]) so each chunk is one big
contiguous DMA on the sync queue; weights ride the scalar queue in 4 large
DMAs; outputs ride the sync queue (idle during the FFN phase).
"""

import sys
import types

sys.path.insert(0, "/opt/trn_rl_repo")

import numpy as np
import ml_dtypes

try:
    import antenv.axon_hooks  # noqa: F401
except ImportError:
    try:
        import antenv
        import trn_agent_boot.trn_boot as _tb

        _hook = _tb._ntff_profile_via_ctypes("/opt/axon/libaxon_pjrt.so")
        _m = types.ModuleType("antenv.axon_hooks")
        _m.get_axon_ntff_profile_hook = lambda: _hook
        _m.set_axon_ntff_profile_hook = lambda h: None
        sys.modules["antenv.axon_hooks"] = _m
        antenv.axon_hooks = _m
    except Exception:
        pass

import concourse.bacc as bacc
import concourse.mybir as mybir
from concourse import bass, bass_utils
from concourse.tile import TileContext
from concourse.masks import make_identity

E = 8
H = 512
F = 2048
T = 8 * 2048
NFT = F // 128  # 16 F-tiles
BF16 = mybir.dt.bfloat16
F32 = mybir.dt.float32

_CACHE = {}
LAST_RESULT = None


def _build(C):
    """Bass program for one core: gating + FFN over C gathered tokens."""
    assert C % 512 == 0
    NG = C // 512
    nc = bacc.Bacc(debug=False)

    xe = nc.declare_dram_parameter("xe", [128, NG, 4, 512], BF16, isOutput=False)
    wg = nc.declare_dram_parameter("wg", [128, 4, E], BF16, isOutput=False)
    bgr = nc.declare_dram_parameter("bgr", [128, E], F32, isOutput=False)
    # w1 F-tile-major: w1[p, ft, c, i] = W1[c*128+p, ft*128+i]
    w1 = nc.declare_dram_parameter("w1", [128, NFT, 4, 128], BF16, isOutput=False)
    b1t = nc.declare_dram_parameter("b1t", [128, NFT], F32, isOutput=False)
    w2 = nc.declare_dram_parameter("w2", [128, NFT, H], BF16, isOutput=False)
    b2r = nc.declare_dram_parameter("b2r", [128, H], F32, isOutput=False)
    # chunk-major output: ypart[g, p, st, :] holds token g*512 + st*128 + p
    # (host transposes back); lets each chunk go out as ONE DMA
    ypart = nc.declare_dram_parameter("ypart", [NG, 128, 4, H], BF16, isOutput=True)

    with TileContext(nc) as tc:
        with (
            tc.tile_pool(name="const", bufs=1) as constp,
            tc.tile_pool(name="xres", bufs=1) as xres,
            tc.tile_pool(name="work", bufs=4) as work,
            tc.tile_pool(name="gate", bufs=3) as gate,
            tc.tile_pool(name="psA", bufs=3, space="PSUM") as psA,
            tc.tile_pool(name="psB", bufs=3, space="PSUM") as psB,
            tc.tile_pool(name="psT", bufs=2, space="PSUM") as psT,
        ):

            # all x chunks parked in SBUF: one large DMA per chunk; chunk 0
            # plus the (tiny) gating params first so gating can start ASAP;
            # later chunks split across both DMA queues
            xall = xres.tile([128, NG, 4, 512], BF16)
            nsync = (NG + 1) // 2 + 1
            nc.sync.dma_start(out=xall[:, 0, :, :], in_=xe[:, 0, :, :])
            wg_sb = constp.tile([128, 4, E], BF16)
            nc.sync.dma_start(out=wg_sb[:], in_=wg[:])
            bgr_sb = constp.tile([128, E], F32)
            nc.sync.dma_start(out=bgr_sb[:], in_=bgr[:])
            w1_sb = constp.tile([128, NFT, 4, 128], BF16)
            for g in range(1, nsync):
                nc.sync.dma_start(out=xall[:, g, :, :], in_=xe[:, g, :, :])
                if g <= 2:
                    # first w1 quarters ride between early x chunks so FFN1
                    # can start the moment gating drains
                    nc.sync.dma_start(
                        out=w1_sb[:, (g - 1) * 4 : g * 4, :, :],
                        in_=w1[:, (g - 1) * 4 : g * 4, :, :],
                    )
            b1_sb = constp.tile([128, NFT], F32)
            nc.scalar.dma_start(out=b1_sb[:], in_=b1t[:])
            for g in range(nsync, NG):
                nc.scalar.dma_start(out=xall[:, g, :, :], in_=xe[:, g, :, :])
            for hf in range(2, 4):
                nc.scalar.dma_start(
                    out=w1_sb[:, hf * 4 : (hf + 1) * 4, :, :],
                    in_=w1[:, hf * 4 : (hf + 1) * 4, :, :],
                )
            b2_sb = constp.tile([128, H], F32)
            nc.scalar.dma_start(out=b2_sb[:], in_=b2r[:])
            w2_sb = constp.tile([128, NFT, H], BF16)
            for hf in range(2):
                nc.scalar.dma_start(
                    out=w2_sb[:, hf * 8 : (hf + 1) * 8, :],
                    in_=w2[:, hf * 8 : (hf + 1) * 8, :],
                )

            lt_all = xres.tile([128, 4 * NG, E], F32)
            comb_all = xres.tile([128, 4 * NG], F32)

            # ---- gating: token-major logits (x subtile stationary, wg
            # moving, N=8 — no transposes needed), ONE Exp for all chunks
            def emit_gate(g):
                for k in range(4):
                    tp = psT.tile([128, E], F32, tag="tp")
                    for c in range(4):
                        nc.tensor.matmul(
                            tp[:],
                            xall[:, g, c, k * 128 : (k + 1) * 128],
                            wg_sb[:, c, :],
                            start=(c == 0),
                            stop=(c == 3),
                        )
                    nc.vector.tensor_tensor(
                        out=lt_all[:, 4 * g + k, :],
                        in0=tp[:],
                        in1=bgr_sb[:],
                        op=mybir.AluOpType.add,
                    )

            def emit_softmax():
                l0 = gate.tile([128, 4 * NG], F32, tag="l0")
                nc.vector.tensor_copy(out=l0[:], in_=lt_all[:, :, 0])
                d = gate.tile([128, 4 * NG, E], F32, tag="d")
                nc.vector.tensor_tensor(
                    out=d[:],
                    in0=lt_all[:],
                    in1=l0[:].to_broadcast([128, 4 * NG, E]),
                    op=mybir.AluOpType.subtract,
                )
                ex = gate.tile([128, 4 * NG, E], F32, tag="ex")
                nc.scalar.activation(ex[:], d[:], mybir.ActivationFunctionType.Exp)
                ssum = gate.tile([128, 4 * NG], F32, tag="ssum")
                nc.vector.tensor_reduce(
                    ssum[:],
                    ex[:],
                    axis=mybir.AxisListType.X,
                    op=mybir.AluOpType.add,
                )
                nc.vector.reciprocal(comb_all[:], ssum[:])

            # ---- FFN (bf16), one Gelu table load, no input DMA
            def emit_ffn1(g):
                hb = work.tile([128, NFT, 512], BF16, tag="hb")
                for ft in range(NFT):
                    hp = psA.tile([128, 512], F32, tag="mmA")
                    for hc in range(4):
                        nc.tensor.matmul(
                            hp[:],
                            w1_sb[:, ft, hc, :],
                            xall[:, g, hc, :],
                            start=(hc == 0),
                            stop=(hc == 3),
                        )
                    nc.scalar.activation(
                        hb[:, ft, :],
                        hp[:],
                        mybir.ActivationFunctionType.Gelu_apprx_tanh,
                        bias=b1_sb[:, ft : ft + 1],
                        scale=1.0,
                    )
                return hb

            # second matmul emitted token-major: lhsT = h tile,
            # moving = W2 rows -> no output transposes needed
            def emit_ffn2(g, hb):
                ybig = work.tile([128, 4, H], BF16, tag="ybig")
                for st in range(4):
                    yp = psB.tile([128, 512], F32, tag="mmB")
                    for fc in range(NFT):
                        nc.tensor.matmul(
                            yp[:],
                            hb[:, fc, st * 128 : (st + 1) * 128],
                            w2_sb[:, fc, :],
                            start=(fc == 0),
                            stop=(fc == NFT - 1),
                        )
                    y32 = work.tile([128, H], F32, tag="y32")
                    nc.vector.tensor_tensor(
                        out=y32[:], in0=yp[:], in1=b2_sb[:], op=mybir.AluOpType.add
                    )
                    nc.vector.tensor_scalar_mul(
                        ybig[:, st, :],
                        y32[:],
                        comb_all[:, 4 * g + st : 4 * g + st + 1],
                    )
                nc.sync.dma_start(out=ypart[g, :, :, :], in_=ybig[:])

            # emission order keeps the in-order PE queue fed: chunk 0's
            # FFN1 (13.8us of matmuls, needs only x0+w1) covers the window
            # while the later x chunks and their gating land
            emit_gate(0)
            emit_gate(1)
            hb0 = emit_ffn1(0)
            for g in range(2, NG):
                emit_gate(g)
            emit_softmax()
            emit_ffn2(0, hb0)
            for g in range(1, NG):
                hb = emit_ffn1(g)
                emit_ffn2(g, hb)
    nc.compile()
    return nc


def _route(xf, Wg, bg):
    """Top-2 routing on host (fp32, same semantics as the reference)."""
    logits = xf @ Wg + bg
    m = logits.max(-1, keepdims=True)
    p = np.exp(logits - m)
    p /= p.sum(-1, keepdims=True)
    order = np.argsort(-p, axis=-1, kind="stable")
    topi = order[:, :2]
    mask = np.zeros_like(p, dtype=bool)
    np.put_along_axis(mask, topi, True, axis=-1)
    idx = [np.nonzero(mask[:, e])[0] for e in range(E)]
    return idx


def _prep_inputs(x, Wg, bg, W1, b1, W2, b2):
    xf = np.ascontiguousarray(np.asarray(x, dtype=np.float32).reshape(T, H))
    Wg = np.asarray(Wg, dtype=np.float32)
    bg = np.asarray(bg, dtype=np.float32)
    W1 = np.asarray(W1, dtype=np.float32)
    b1 = np.asarray(b1, dtype=np.float32)
    W2 = np.asarray(W2, dtype=np.float32)
    b2 = np.asarray(b2, dtype=np.float32)

    idx = _route(xf, Wg, bg)
    maxc = max(len(i) for i in idx)
    C = max(512, -(-maxc // 512) * 512)
    NG = C // 512

    # x transposed to [128, 4, T] (h = c*128 + p), bf16
    xbt = np.ascontiguousarray(
        np.transpose(xf.T.reshape(4, 128, T), (1, 0, 2))
    ).astype(ml_dtypes.bfloat16)

    in_maps = []
    for e in range(E):
        xg = np.zeros((128, 4, C), dtype=ml_dtypes.bfloat16)
        xg[:, :, : len(idx[e])] = xbt[:, :, idx[e]]
        # chunk-major: [128, g, c, 512]
        xg = np.ascontiguousarray(
            np.transpose(xg.reshape(128, 4, NG, 512), (0, 2, 1, 3))
        )
        perm = [e] + [j for j in range(E) if j != e]
        wg_p = Wg[:, perm]
        bg_p = bg[perm]
        # w1 F-tile-major: [128, ft, c, i] = W1[c*128+p, ft*128+i]
        w1r = np.ascontiguousarray(
            np.transpose(
                W1[e].reshape(4, 128, NFT, 128), (1, 2, 0, 3)
            ).astype(ml_dtypes.bfloat16)
        )
        in_maps.append(
            {
                "xe": xg,
                "wg": np.ascontiguousarray(
                    np.transpose(wg_p.reshape(4, 128, E), (1, 0, 2))
                ).astype(ml_dtypes.bfloat16),
                "bgr": np.ascontiguousarray(
                    np.broadcast_to(bg_p[None, :], (128, E)).copy()
                ),
                "w1": w1r,
                "b1t": np.ascontiguousarray(b1[e].reshape(NFT, 128).T),
                "w2": np.ascontiguousarray(
                    np.transpose(W2[e].reshape(NFT, 128, H), (1, 0, 2)).astype(
                        ml_dtypes.bfloat16
                    )
                ),
                "b2r": np.ascontiguousarray(
                    np.broadcast_to(b2[e][None, :], (128, H)).copy()
                ),
            }
        )
    return in_maps, idx, C


def kernel(x, Wg, bg, W1, b1, W2, b2):
    global LAST_RESULT
    in_maps, idx, C = _prep_inputs(x, Wg, bg, W1, b1, W2, b2)
    if C not in _CACHE:
        _CACHE[C] = _build(C)
    nc = _CACHE[C]
    import os

    trace = bool(os.environ.get("BASS_TRACE"))
    res = bass_utils.run_bass_kernel_spmd(
        nc, in_maps, core_ids=list(range(E)), trace=trace
    )
    LAST_RESULT = res
    NG = C // 512
    out = np.zeros((T, H), dtype=np.float32)
    for e in range(E):
        # decode chunk-major [NG, 128, 4, H] -> token-order [C, H]
        yp = np.transpose(res.results[e]["ypart"], (0, 2, 1, 3)).reshape(C, H)
        out[idx[e]] += yp[: len(idx[e])].astype(np.float32)
    return out.reshape(8, 2048, H)
